# revision 1
# baseline (speedup 1.0000x reference)
"""Cross-attention kernel for one TRN2 chip (8 NeuronCores).

Sharding: core = (batch b in {0,1}) x (head-group of 4 heads).  Each core
computes attention for its 4 heads of its batch element and a partial output
projection [N, 1024]; the host sums the 4 partials per batch and adds the
bias.

Layout strategy per core (all matmuls bf16 with fp32 PSUM accumulation):
  xT/cT  [c=1024 (8 p-chunks), seq 2048]   via cast-DMA + SBUF->SBUF DMA transpose
  qT/kT  [d'=256 (2 p-chunks), seq 2048]   from projection (weights stationary)
  v      [m, 4 heads, 64+1]                natural layout, ones column appended so
                                           the AV matmul also produces the softmax
                                           denominator (no separate reduction)
  scores sT [m-tile 128, n 2048] in PSUM; exp on ScalarE (scale=1/8 folded in,
  no max subtraction -- scores are O(1) for this problem); AV accumulates
  oT [65, n-chunk] over m-tiles; normalization is deferred to after AV.
"""

import numpy as np

import concourse.bass as bass
import concourse.mybir as mybir
import concourse.tile as tile
from concourse import bacc
from concourse.bass import ts
from concourse.bass_utils import run_bass_kernel_spmd
from concourse.masks import make_identity

B, N, M, C = 2, 2048, 2048, 1024
HEADS, DH = 16, 64
H_PER = 4                # heads per core
DHC = H_PER * DH         # 256: per-core slice of INNER
SCALE = DH ** -0.5
P = 128
NT = N // P              # 16 n-tiles
MT = M // P              # 16 m-tiles
CCH = C // P             # 8 contraction chunks
FD = 512                 # matmul moving free dim
NCH = N // FD            # 4 n-chunks
N_CORES = 8

F32 = mybir.dt.float32
BF16 = mybir.dt.bfloat16
EXP = mybir.ActivationFunctionType.Exp

_CACHE = {}


def _build():
    nc = bacc.Bacc("TRN2", target_bir_lowering=False, debug=False,
                   num_devices=N_CORES, num_swdge_queues=4)

    x_d = nc.dram_tensor("x", (N, C), F32, kind="ExternalInput").ap()
    ctx_d = nc.dram_tensor("ctx", (M, C), F32, kind="ExternalInput").ap()
    msk_d = nc.dram_tensor("msk", (M, 1), F32, kind="ExternalInput").ap()
    wq_d = nc.dram_tensor("wq", (C, DHC), F32, kind="ExternalInput").ap()
    wk_d = nc.dram_tensor("wk", (C, DHC), F32, kind="ExternalInput").ap()
    wv_d = nc.dram_tensor("wv", (C, DHC), F32, kind="ExternalInput").ap()
    wo_d = nc.dram_tensor("wo", (DHC, C), F32, kind="ExternalInput").ap()
    y_d = nc.dram_tensor("y", (N, C), F32, kind="ExternalOutput").ap()

    with tile.TileContext(nc) as tc:
        with (
            tc.tile_pool(name="const", bufs=1) as const,
            tc.tile_pool(name="stage", bufs=8) as stage,
            tc.tile_pool(name="pTp", bufs=6) as pTp,
            tc.tile_pool(name="norm", bufs=3) as norm,
            tc.tile_pool(name="yp", bufs=3) as yp,
            tc.tile_pool(name="dramp", bufs=2, space="DRAM") as dramp,
        ):
            # ---- persistent SBUF tensors ----
            xT = [const.tile([P, N], BF16, name=f"xT{cc}") for cc in range(CCH)]
            cT = [const.tile([P, M], BF16, name=f"cT{cc}") for cc in range(CCH)]
            qT = [const.tile([P, N], BF16, name=f"qT{dc}") for dc in range(2)]
            kT = [const.tile([P, M], BF16, name=f"kT{dc}") for dc in range(2)]
            oTp = [const.tile([P, N], BF16, name=f"oTp{dc}") for dc in range(2)]
            v_sb = [const.tile([P, H_PER, DH + 1], BF16, name=f"v{m}")
                    for m in range(MT)]
            wq_sb = const.tile([P, CCH, DHC], BF16, name="wq")
            wk_sb = const.tile([P, CCH, DHC], BF16, name="wk")
            wv_sb = const.tile([P, CCH, DHC], BF16, name="wv")
            wo_sb = const.tile([P, 2, C], BF16, name="wo")
            msk_sb = const.tile([P, MT, 1], F32, name="msk")

            # ---- weights + mask (cast f32 -> bf16 via SWDGE) ----
            nc.gpsimd.dma_start(
                out=wk_sb, in_=wk_d.rearrange("(cc p) d -> p cc d", p=P))
            nc.gpsimd.dma_start(
                out=wv_sb, in_=wv_d.rearrange("(cc p) d -> p cc d", p=P))
            nc.gpsimd.dma_start(
                out=wq_sb, in_=wq_d.rearrange("(cc p) d -> p cc d", p=P))
            nc.gpsimd.dma_start(
                out=wo_sb, in_=wo_d.rearrange("(dc p) e -> p dc e", p=P))
            nc.sync.dma_start(
                out=msk_sb, in_=msk_d.rearrange("(t p) o -> p t o", p=P))

            ident = const.tile([P, P], F32, name="ident")
            make_identity(nc, ident)

            ps_proj_cm = tc.tile_pool(name="ps_proj", bufs=2, space="PSUM")
            ps_proj = ps_proj_cm.__enter__()

            # cast-load a row-tile, PE-transpose each 128x128 block, copy-cast
            # to the channel-major SBUF tensor (copies split DVE/ACT)
            def load_T(src_ap, dstT, t):
                st = stage.tile([P, C], F32, name="stage")
                nc.sync.dma_start(out=st, in_=src_ap[ts(t, P), :])
                for cc in range(CCH):
                    tp = ps_proj.tile([P, P], F32, name="tp")
                    nc.tensor.transpose(tp, st[:, ts(cc, P)], ident)
                    if cc % 2 == 0:
                        nc.vector.tensor_copy(dstT[cc][:, ts(t, P)], tp)
                    else:
                        nc.scalar.copy(dstT[cc][:, ts(t, P)], tp)

            def proj_T(w_sb, src_T, dst_T, dc, j):
                ps = ps_proj.tile([P, FD], F32, name="kq")
                for cc in range(CCH):
                    nc.tensor.matmul(
                        ps, lhsT=w_sb[:, cc, ts(dc, P)],
                        rhs=src_T[cc][:, ts(j, FD)],
                        start=(cc == 0), stop=(cc == CCH - 1))
                nc.vector.tensor_copy(dst_T[dc][:, ts(j, FD)], ps)

            # ctx pipeline: 4 row-tiles then the K-proj chunk they enable
            for g in range(4):
                for t in range(4 * g, 4 * g + 4):
                    load_T(ctx_d, cT, t)
                proj_T(wk_sb, cT, kT, 0, g)

            # ---- V projection: natural layout + ones column + mask ----
            for m in range(MT):
                vp = ps_proj.tile([P, DHC], F32, name="vp")
                for cc in range(CCH):
                    nc.tensor.matmul(
                        vp, lhsT=cT[cc][:, ts(m, P)], rhs=wv_sb[:, cc, :],
                        start=(cc == 0), stop=(cc == CCH - 1))
                nc.vector.memset(v_sb[m], 1.0)
                nc.vector.tensor_copy(
                    v_sb[m][:, :, 0:DH],
                    vp.rearrange("p (h d) -> p h d", h=H_PER))
                nc.vector.tensor_scalar_mul(v_sb[m], v_sb[m], msk_sb[:, m, :])

            # x pipeline + Q-proj chunks
            for g in range(4):
                for t in range(4 * g, 4 * g + 4):
                    load_T(x_d, xT, t)
                proj_T(wq_sb, xT, qT, 0, g)
            for g in range(4):
                proj_T(wk_sb, cT, kT, 1, g)
            for g in range(4):
                proj_T(wq_sb, xT, qT, 1, g)
            ps_proj_cm.__exit__(None, None, None)

            # ---- attention: head pairs (row-group packed QK), n-half
            # passes to fit PSUM (sT 2x2 banks + oT 4 banks) ----
            ps_sT_cm = tc.tile_pool(name="ps_sT", bufs=1, space="PSUM")
            ps_sT = ps_sT_cm.__enter__()
            ps_oT_cm = tc.tile_pool(name="ps_oT", bufs=1, space="PSUM")
            ps_oT = ps_oT_cm.__enter__()
            for dc in range(2):
                for pf in range(2):
                    oT = {}
                    for s in range(2):
                        for jj in range(2):
                            oT[(s, jj)] = ps_oT.tile(
                                [DH + 1, FD], F32, name=f"oT{s}{jj}")
                    for m in range(MT):
                        sTs = []
                        for s in range(2):
                            sT = ps_sT.tile([P, N // 2], F32, name=f"sT{s}")
                            for jj in range(2):
                                j = pf * 2 + jj
                                nc.tensor.matmul(
                                    sT[:, ts(jj, FD)],
                                    lhsT=kT[dc][s * DH:(s + 1) * DH, ts(m, P)],
                                    rhs=qT[dc][s * DH:(s + 1) * DH, ts(j, FD)],
                                    start=True, stop=True)
                            sTs.append(sT)
                        for s in range(2):
                            pT = pTp.tile([P, N // 2], BF16, name=f"pT{s}")
                            nc.scalar.activation(pT, sTs[s], EXP, scale=SCALE)
                            for jj in range(2):
                                nc.tensor.matmul(
                                    oT[(s, jj)],
                                    lhsT=v_sb[m][:, 2 * dc + s, :],
                                    rhs=pT[:, ts(jj, FD)],
                                    start=(m == 0), stop=(m == MT - 1))
                    # normalize: divide by the ones-column sums, pack into oTp
                    for s in range(2):
                        for jj in range(2):
                            j = pf * 2 + jj
                            o_f = norm.tile([DH + 1, FD], F32, name="o_f")
                            nc.vector.tensor_copy(o_f, oT[(s, jj)])
                            nc.vector.reciprocal(
                                o_f[DH:DH + 1, :], o_f[DH:DH + 1, :])
                            sums_d = dramp.tile([1, FD], F32, name="sums_d")
                            nc.sync.dma_start(
                                out=sums_d, in_=o_f[DH:DH + 1, :])
                            rec = norm.tile([DH, FD], F32, name="rec")
                            nc.gpsimd.dma_start(
                                out=rec, in_=sums_d.to_broadcast((DH, FD)))
                            if s == 0:
                                nc.vector.tensor_mul(
                                    oTp[dc][0:DH, ts(j, FD)], o_f[0:DH, :],
                                    rec)
                            else:
                                ob = norm.tile([DH, FD], BF16, name="ob")
                                nc.vector.tensor_mul(ob, o_f[0:DH, :], rec)
                                nc.sync.dma_start(
                                    out=oTp[dc][DH:2 * DH, ts(j, FD)], in_=ob)
            ps_oT_cm.__exit__(None, None, None)
            ps_sT_cm.__exit__(None, None, None)

            # ---- output projection ----
            ps_y_cm = tc.tile_pool(name="ps_y", bufs=2, space="PSUM")
            ps_y = ps_y_cm.__enter__()
            for i in range(NT):
                y_ps = ps_y.tile([P, C], F32, name="y")
                for dc in range(2):
                    for col in range(2):
                        nc.tensor.matmul(
                            y_ps[:, ts(col, FD)],
                            lhsT=oTp[dc][:, ts(i, P)],
                            rhs=wo_sb[:, dc, ts(col, FD)],
                            start=(dc == 0), stop=(dc == 1))
                y_sb = yp.tile([P, C], F32, name="ysb")
                if i % 2 == 0:
                    nc.vector.tensor_copy(y_sb, y_ps)
                else:
                    nc.scalar.copy(y_sb, y_ps)
                nc.sync.dma_start(out=y_d[ts(i, P), :], in_=y_sb)
            ps_y_cm.__exit__(None, None, None)

    nc.compile()
    return nc


def _in_maps(x, context, mask, Wq, Wk, Wv, Wo):
    maps = []
    for core in range(N_CORES):
        b, hg = core // H_PER, core % H_PER
        c0 = hg * DHC
        maps.append({
            "x": np.ascontiguousarray(x[b], dtype=np.float32),
            "ctx": np.ascontiguousarray(context[b], dtype=np.float32),
            "msk": np.ascontiguousarray(
                mask[b].astype(np.float32).reshape(M, 1)),
            "wq": np.ascontiguousarray(Wq[:, c0:c0 + DHC], dtype=np.float32),
            "wk": np.ascontiguousarray(Wk[:, c0:c0 + DHC], dtype=np.float32),
            "wv": np.ascontiguousarray(Wv[:, c0:c0 + DHC], dtype=np.float32),
            "wo": np.ascontiguousarray(Wo[c0:c0 + DHC, :], dtype=np.float32),
        })
    return maps


def _gather(results, bo):
    out = np.zeros((B, N, C), dtype=np.float32)
    for core in range(N_CORES):
        out[core // H_PER] += results[core]["y"]
    out += np.asarray(bo, dtype=np.float32)
    return out


def kernel(x, context, mask, Wq, Wk, Wv, Wo, bo, **extra_kwargs):
    if "nc" not in _CACHE:
        _CACHE["nc"] = _build()
    nc = _CACHE["nc"]
    maps = _in_maps(x, context, mask, Wq, Wk, Wv, Wo)
    res = run_bass_kernel_spmd(nc, maps, core_ids=list(range(N_CORES)),
                               **extra_kwargs)
    out = _gather(res.results, bo)
    if extra_kwargs:
        _CACHE["last_result"] = res
    return out



# revision 8
# speedup vs baseline: 1.5189x; 1.5189x over previous
"""Cross-attention kernel for one TRN2 chip (8 NeuronCores).

Sharding: core = (batch b in {0,1}) x (head-group of 4 heads).  Each core
computes attention for its 4 heads of its batch element and a partial output
projection [N, 1024]; the host sums the 4 partials per batch and adds bias.

Key structure (all matmuls bf16, fp32 PSUM):
  - x/ctx cast-loaded to bf16 (SWDGE), transposed on the PE via regular
    matmuls against a bf16 identity (keeps HAM warm; ~2x faster than
    transpose-mode).
  - QK per m-tile: two concurrent row-tiled matmuls (head s0 on array rows
    0-63, s1 on rows 64-127) into one [128,1024] PSUM tile; ONE wide exp
    [128,1024] on ScalarE covers both heads; AV accumulates [65,512] per
    head with a ones-column producing the softmax denominator for free.
  - PSUM: sT double-buffered (4 banks) + oT double-buffered (4 banks) so
    the PE never stalls on the activation and HAM stays at 2.4 GHz.
  - Normalization: denominators go PSUM->SBUF->DRAM, are gathered into a
    [128,64] tile, reciprocal_approx_accurate, scattered back, broadcast-
    DMA'd across partitions, one tensor_mul per d-chunk.
"""

import numpy as np

import concourse.bass as bass
import concourse.mybir as mybir
import concourse.tile as tile
from concourse import bacc
from concourse.bass import ts
from concourse.bass_utils import run_bass_kernel_spmd
from concourse.masks import make_identity

B, N, M, C = 2, 2048, 2048, 1024
HEADS, DH = 16, 64
H_PER = 4                # heads per core
DHC = H_PER * DH         # 256: per-core slice of INNER
SCALE = DH ** -0.5
P = 128
NT = N // P              # 16 n-tiles
MT = M // P              # 16 m-tiles
CCH = C // P             # 8 contraction chunks
FD = 512                 # attention n-chunk (PSUM bank)
NJ = N // FD             # 4 n-chunks
N_CORES = 8

F32 = mybir.dt.float32
BF16 = mybir.dt.bfloat16
EXP = mybir.ActivationFunctionType.Exp

_CACHE = {}
DEBUG_PROBES = False


def _build():
    nc = bacc.Bacc("TRN2", target_bir_lowering=False, debug=False,
                   num_devices=N_CORES, num_swdge_queues=4)

    x_d = nc.dram_tensor("x", (N, C), F32, kind="ExternalInput").ap()
    ctx_d = nc.dram_tensor("ctx", (M, C), F32, kind="ExternalInput").ap()
    msk_d = nc.dram_tensor("msk", (M, 1), F32, kind="ExternalInput").ap()
    wq_d = nc.dram_tensor("wq", (C, DHC), F32, kind="ExternalInput").ap()
    wk_d = nc.dram_tensor("wk", (C, DHC), F32, kind="ExternalInput").ap()
    wv_d = nc.dram_tensor("wv", (C, DHC), F32, kind="ExternalInput").ap()
    wo_d = nc.dram_tensor("wo", (DHC, C), F32, kind="ExternalInput").ap()
    y_d = nc.dram_tensor("y", (N, C), F32, kind="ExternalOutput").ap()
    if DEBUG_PROBES:
        dbg_dg = nc.dram_tensor("dbg_dg", (2, P, 32), F32,
                                kind="ExternalOutput").ap()
        dbg_r = nc.dram_tensor("dbg_r", (2, P, 32), F32,
                               kind="ExternalOutput").ap()
        dbg_rb = nc.dram_tensor("dbg_rb", (P, 2, N), F32,
                                kind="ExternalOutput").ap()
        dbg_otu = nc.dram_tensor("dbg_otu", (P, 2, N), F32,
                                 kind="ExternalOutput").ap()

    with tile.TileContext(nc) as tc:
        with (
            tc.tile_pool(name="const", bufs=1) as const,
            tc.tile_pool(name="stage", bufs=6) as stage,
            tc.tile_pool(name="pTp", bufs=3) as pTp,
            tc.tile_pool(name="drn", bufs=3) as drn,
            tc.tile_pool(name="norm", bufs=1) as norm,
            tc.tile_pool(name="yp", bufs=3) as yp,
            tc.tile_pool(name="dramp", bufs=1, space="DRAM") as dramp,
        ):
            # ---- persistent SBUF tensors ----
            xT = const.tile([P, CCH, N], BF16, name="xT")
            cT = const.tile([P, CCH, M], BF16, name="cT")
            qT2 = const.tile([P, 2, N], BF16, name="qT2")
            kT2 = const.tile([P, 2, M], BF16, name="kT2")
            # v: [m-partition, m-tile, head, d(64)+ones(1)]
            v_sb = const.tile([P, MT, H_PER, DH + 1], BF16, name="v")
            wq_sb = const.tile([P, CCH, DHC], BF16, name="wq")
            wk_sb = const.tile([P, CCH, DHC], BF16, name="wk")
            wv_sb = const.tile([P, CCH, DHC], BF16, name="wv")
            wo_sb = const.tile([P, 2, C], BF16, name="wo")
            msk_sb = const.tile([P, MT, 1], F32, name="msk")
            oTu = const.tile([P, 2, N], BF16, name="oTu")    # un-normalized
            oTn = const.tile([P, 2, N], BF16, name="oTn")    # normalized
            Rb = const.tile([P, 2, N], F32, name="Rb")       # 1/D broadcast

            # ---- weights + mask (cast f32 -> bf16 via SWDGE) ----
            nc.gpsimd.dma_start(
                out=wk_sb, in_=wk_d.rearrange("(cc p) d -> p cc d", p=P))
            nc.gpsimd.dma_start(
                out=wv_sb, in_=wv_d.rearrange("(cc p) d -> p cc d", p=P))
            nc.gpsimd.dma_start(
                out=wq_sb, in_=wq_d.rearrange("(cc p) d -> p cc d", p=P))
            nc.gpsimd.dma_start(
                out=wo_sb, in_=wo_d.rearrange("(dc p) e -> p dc e", p=P))
            nc.sync.dma_start(
                out=msk_sb, in_=msk_d.rearrange("(t p) o -> p t o", p=P))

            identf = stage.tile([P, P], F32, name="identf")
            make_identity(nc, identf)
            identb = const.tile([P, P], BF16, name="identb")
            nc.vector.tensor_copy(identb, identf)

            nc.vector.memset(v_sb, 1.0)

            ps_t_cm = tc.tile_pool(name="ps_t", bufs=2, space="PSUM")
            ps_t = ps_t_cm.__enter__()       # [128,1024] transposes: 4 banks
            ps_p_cm = tc.tile_pool(name="ps_p", bufs=2, space="PSUM")
            ps_p = ps_p_cm.__enter__()       # [128,512] projections: 2 banks

            # cast-load one row-tile and PE-transpose it into dstT[:, :, t*P]
            def load_T(src_ap, src_sb_name, dstT, t, alt):
                sn = stage.tile([P, C], BF16, name=src_sb_name)
                nc.gpsimd.dma_start(out=sn, in_=src_ap[ts(t, P), :])
                tp = ps_t.tile([P, C], F32, name="tp")
                for cc in range(CCH):
                    nc.tensor.matmul(
                        tp[:, ts(cc, P)], lhsT=sn[:, ts(cc, P)], rhs=identb,
                        start=True, stop=True)
                dst = dstT[:, :, ts(t, P)]
                src = tp.rearrange("p (cc n) -> p cc n", cc=CCH)
                if alt:
                    nc.vector.tensor_copy(dst, src)
                else:
                    nc.scalar.copy(dst, src)

            # project one n/m-chunk j of q or k (both d-chunks dc)
            def proj_T(w_sb, srcT, dstT2, dc, j, alt):
                ps = ps_p.tile([P, FD], F32, name="kq")
                for cc in range(CCH):
                    nc.tensor.matmul(
                        ps, lhsT=w_sb[:, cc, ts(dc, P)],
                        rhs=srcT[:, cc, ts(j, FD)],
                        start=(cc == 0), stop=(cc == CCH - 1))
                dst = dstT2[:, dc, ts(j, FD)]
                if alt:
                    nc.vector.tensor_copy(dst, ps)
                else:
                    nc.scalar.copy(dst, ps)

            # V projection for two m-tiles (one [128,512] PSUM tile)
            def proj_V(m0):
                vp = ps_p.tile([P, 2, DHC], F32, name="vp")
                for mi in range(2):
                    for cc in range(CCH):
                        nc.tensor.matmul(
                            vp[:, mi, :],
                            lhsT=cT[:, cc, ts(m0 + mi, P)],
                            rhs=wv_sb[:, cc, :],
                            start=(cc == 0), stop=(cc == CCH - 1))
                nc.vector.tensor_copy(
                    v_sb[:, m0:m0 + 2, :, 0:DH],
                    vp.rearrange("p mi (h d) -> p mi h d", h=H_PER))
                for mi in range(2):
                    nc.vector.tensor_scalar_mul(
                        v_sb[:, m0 + mi, :, :], v_sb[:, m0 + mi, :, :],
                        msk_sb[:, m0 + mi, :])

            # ---- phase A: ctx pipeline (transpose + K/V), then x (+Q) ----
            alt = 0
            for g in range(4):
                for t in range(4 * g, 4 * g + 4):
                    load_T(ctx_d, "cn", cT, t, alt % 2)
                    alt += 1
            for g in range(4):
                for dc in range(2):
                    proj_T(wk_sb, cT, kT2, dc, g, alt % 2)
                    alt += 1
                proj_V(4 * g)
                proj_V(4 * g + 2)
            for g in range(4):
                for t in range(4 * g, 4 * g + 4):
                    load_T(x_d, "xn", xT, t, alt % 2)
                    alt += 1
            for g in range(4):
                for dc in range(2):
                    proj_T(wq_sb, xT, qT2, dc, g, alt % 2)
                    alt += 1

            ps_p_cm.__exit__(None, None, None)
            ps_t_cm.__exit__(None, None, None)

            # ---- denominators collect tile in DRAM ----
            Dc = dramp.tile([2, 2, NJ, FD], F32, name="Dc")
            Rd = dramp.tile([2, 2, N], F32, name="Rd")

            # ---- phase B: attention ----
            ps_s_cm = tc.tile_pool(name="ps_s", bufs=2, space="PSUM")
            ps_s = ps_s_cm.__enter__()       # [128,1024] scores: 4 banks
            ps_o_cm = tc.tile_pool(name="ps_o", bufs=2, space="PSUM")
            ps_o = ps_o_cm.__enter__()       # [65,1024] out accum: 4 banks

            def qk(sT, dc, j, m):
                for s in range(2):
                    nc.tensor.matmul(
                        sT[:, s, :],
                        lhsT=kT2[s * DH:(s + 1) * DH, dc, ts(m, P)],
                        rhs=qT2[s * DH:(s + 1) * DH, dc, ts(j, FD)],
                        start=True, stop=True)

            def av(oT, pT, dc, m):
                for s in range(2):
                    nc.tensor.matmul(
                        oT[:, s, :],
                        lhsT=v_sb[:, m, 2 * dc + s, :],
                        rhs=pT[:, s, :],
                        start=(m == 0), stop=(m == MT - 1))

            for dc in range(2):
                for j in range(NJ):
                    oT = ps_o.tile([DH + 1, 2, FD], F32, name="oT")
                    pT_prev = None
                    for m in range(MT):
                        sT = ps_s.tile([P, 2, FD], F32, name="sT")
                        qk(sT, dc, j, m)
                        if pT_prev is not None:
                            av(oT, pT_prev, dc, m - 1)
                        pT = pTp.tile([P, 2, FD], BF16, name="pT")
                        nc.scalar.activation(pT, sT, EXP, scale=SCALE)
                        pT_prev = pT
                    av(oT, pT_prev, dc, MT - 1)
                    # drain: s0 direct; s1 via stage + partition-shift DMA;
                    # D rows (row 64 of both halves) to DRAM in fp32
                    nc.vector.tensor_copy(
                        oTu[0:DH, dc, ts(j, FD)], oT[0:DH, 0, :])
                    st1 = drn.tile([DH, FD], BF16, name="st1")
                    nc.vector.tensor_copy(st1, oT[0:DH, 1, :])
                    nc.sync.dma_start(
                        out=oTu[DH:2 * DH, dc, ts(j, FD)], in_=st1)
                    dst = drn.tile([1, 2, FD], F32, name="dst")
                    nc.vector.tensor_copy(dst, oT[DH:DH + 1, :, :])
                    nc.sync.dma_start(out=Dc[dc, :, j, :], in_=dst)

                # normalization chain for this dc (overlaps next dc's work)
                Dg = norm.tile([P, 2 * NJ * 4], F32, name=f"Dg{dc}")
                nc.sync.dma_start(
                    out=Dg,
                    in_=Dc[dc].rearrange("s j (w p) -> p (s j w)", p=P))
                R = norm.tile([P, 2 * NJ * 4], F32, name=f"R{dc}")
                nc.vector.reciprocal(R, Dg)
                nc.sync.dma_start(
                    out=Rd[dc].rearrange("s (j w p) -> p (s j w)", p=P, w=4),
                    in_=R)
                for s in range(2):
                    nc.gpsimd.dma_start(
                        out=Rb[s * DH:(s + 1) * DH, dc, :],
                        in_=Rd[dc, s:s + 1, :].to_broadcast((DH, N)))
                nc.vector.tensor_mul(
                    oTn[:, dc, :], oTu[:, dc, :], Rb[:, dc, :])
                if DEBUG_PROBES:
                    nc.sync.dma_start(out=dbg_dg[dc], in_=Dg)
                    nc.sync.dma_start(out=dbg_r[dc], in_=R)

            if DEBUG_PROBES:
                nc.sync.dma_start(out=dbg_rb, in_=Rb)
                nc.gpsimd.dma_start(out=dbg_otu, in_=oTu)

            ps_o_cm.__exit__(None, None, None)
            ps_s_cm.__exit__(None, None, None)

            # ---- phase C: output projection ----
            ps_y_cm = tc.tile_pool(name="ps_y", bufs=2, space="PSUM")
            ps_y = ps_y_cm.__enter__()
            for i in range(NT):
                y_ps = ps_y.tile([P, C], F32, name="y")
                for col in range(2):
                    for dc in range(2):
                        nc.tensor.matmul(
                            y_ps[:, ts(col, FD)],
                            lhsT=oTn[:, dc, ts(i, P)],
                            rhs=wo_sb[:, dc, ts(col, FD)],
                            start=(dc == 0), stop=(dc == 1))
                y_sb = yp.tile([P, C], F32, name="ysb")
                if i % 2 == 0:
                    nc.vector.tensor_copy(y_sb, y_ps)
                else:
                    nc.scalar.copy(y_sb, y_ps)
                nc.sync.dma_start(out=y_d[ts(i, P), :], in_=y_sb)
            ps_y_cm.__exit__(None, None, None)

    nc.compile()
    return nc


def _in_maps(x, context, mask, Wq, Wk, Wv, Wo):
    maps = []
    for core in range(N_CORES):
        b, hg = core // H_PER, core % H_PER
        c0 = hg * DHC
        maps.append({
            "x": np.ascontiguousarray(x[b], dtype=np.float32),
            "ctx": np.ascontiguousarray(context[b], dtype=np.float32),
            "msk": np.ascontiguousarray(
                mask[b].astype(np.float32).reshape(M, 1)),
            "wq": np.ascontiguousarray(Wq[:, c0:c0 + DHC], dtype=np.float32),
            "wk": np.ascontiguousarray(Wk[:, c0:c0 + DHC], dtype=np.float32),
            "wv": np.ascontiguousarray(Wv[:, c0:c0 + DHC], dtype=np.float32),
            "wo": np.ascontiguousarray(Wo[c0:c0 + DHC, :], dtype=np.float32),
        })
    return maps


def _gather(results, bo):
    out = np.zeros((B, N, C), dtype=np.float32)
    for core in range(N_CORES):
        out[core // H_PER] += results[core]["y"]
    out += np.asarray(bo, dtype=np.float32)
    return out


def kernel(x, context, mask, Wq, Wk, Wv, Wo, bo, **extra_kwargs):
    if "nc" not in _CACHE:
        _CACHE["nc"] = _build()
    nc = _CACHE["nc"]
    maps = _in_maps(x, context, mask, Wq, Wk, Wv, Wo)
    res = run_bass_kernel_spmd(nc, maps, core_ids=list(range(N_CORES)),
                               **extra_kwargs)
    out = _gather(res.results, bo)
    if extra_kwargs:
        _CACHE["last_result"] = res
    return out
